# revision 3
# baseline (speedup 1.0000x reference)
"""CZ-ring (12 wires) applied to a batch of states: y = U @ x.

Every gate in the ring is a controlled-Z, which is diagonal in the
computational basis: CZ(c,t) = diag((-1)^(b_c & b_t)).  The product of
the 12 ring CZ gates is therefore also diagonal:

    U = diag(d),   d[b] = (-1)^(sum_i b_i * b_{(i+1) mod 12})

so U @ x is just a per-row sign flip of x.  Of the 4096 rows, 2112
have d=+1 and 1984 have d=-1.  We shard rows across the 8 cores with a
host-side permutation that gives every core the same layout: 264 "+"
rows followed by 248 "-" rows.  On device the "+" block is a single
DRAM->DRAM DMA copy and the "-" block is one load -> multiply by the
immediate -1.0 -> store.  Each core moves ~2 MiB in + ~2 MiB out, so
the kernel sits at the per-core HBM-bandwidth roofline.
"""

import numpy as np

N_WIRES = 12
DIM = 1 << N_WIRES  # 4096
BATCH = 1024
N_CORES = 8
ROWS_PER_CORE = DIM // N_CORES  # 512
PLUS_PER_CORE = 264  # 2112 / 8
MINUS_PER_CORE = 248  # 1984 / 8
MINUS_P = 124  # minus block as [124 partitions, 2, 1024]

_cache: dict = {}


def _sign_parity() -> np.ndarray:
    """parity[b] = sum_i b_i * b_{(i+1) mod N_WIRES} mod 2  (1 => d=-1)."""
    b = np.arange(DIM, dtype=np.uint32)
    parity = np.zeros(DIM, dtype=np.uint32)
    for i in range(N_WIRES):
        bi = (b >> np.uint32(i)) & np.uint32(1)
        bj = (b >> np.uint32((i + 1) % N_WIRES)) & np.uint32(1)
        parity ^= bi & bj
    return parity


def _row_assignment():
    """Per-core row index lists: 264 plus rows then 248 minus rows each."""
    parity = _sign_parity()
    plus_rows = np.nonzero(parity == 0)[0]  # 2112
    minus_rows = np.nonzero(parity == 1)[0]  # 1984
    assert len(plus_rows) == PLUS_PER_CORE * N_CORES
    assert len(minus_rows) == MINUS_PER_CORE * N_CORES
    perms = []
    for k in range(N_CORES):
        p = plus_rows[k * PLUS_PER_CORE : (k + 1) * PLUS_PER_CORE]
        m = minus_rows[k * MINUS_PER_CORE : (k + 1) * MINUS_PER_CORE]
        perms.append(np.concatenate([p, m]))
    return perms


def _build_program():
    from concourse import bass
    import concourse.mybir as mybir

    f32 = mybir.dt.float32
    nc = bass.Bass("TRN2", target_bir_lowering=False, debug=False)
    x_in = nc.dram_tensor("x", [ROWS_PER_CORE, BATCH], f32, kind="ExternalInput").ap()
    y_out = nc.dram_tensor(
        "y", [ROWS_PER_CORE, BATCH], f32, kind="ExternalOutput"
    ).ap()
    t = nc.alloc_sbuf_tensor("t", [MINUS_P, 2, BATCH], f32).ap()

    x_minus = x_in[PLUS_PER_CORE:, :].rearrange("(n p) d -> p n d", p=MINUS_P)
    y_minus = y_out[PLUS_PER_CORE:, :].rearrange("(n p) d -> p n d", p=MINUS_P)

    # Raw bass (no TileContext): the tile scheduler's tail Drain collects one
    # sem wait per DMA lane + engine and overflows this toolchain's
    # per-instruction sync-wait budget; explicit standalone waits keep every
    # instruction at <=1 wait.
    with (
        nc.Block() as block,
        nc.semaphore("ld_sem") as ld_sem,
        nc.semaphore("cp_sem") as cp_sem,
        nc.semaphore("dve_sem") as dve_sem,
        nc.semaphore("st_sem") as st_sem,
    ):

        @block.sync
        def _(sync: bass.BassEngine):
            # "-" rows: load first (the negate depends on it) ...
            sync.dma_start(out=t[:, :, :], in_=x_minus).then_inc(ld_sem, 16)
            # ... then the "+" rows: identity -> straight DRAM->DRAM copy
            sync.dma_start(
                out=y_out[:PLUS_PER_CORE, :], in_=x_in[:PLUS_PER_CORE, :]
            ).then_inc(cp_sem, 16)
            sync.wait_ge(cp_sem, 16)

        @block.vector
        def _(vector: bass.BassEngine):
            vector.wait_ge(ld_sem, 16)
            vector.tensor_scalar_mul(t[:, :, :], t[:, :, :], -1.0).then_inc(
                dve_sem, 1
            )

        @block.scalar
        def _(scalar: bass.BassEngine):
            # store on the ACT HWDGE ring so it overlaps the SP-ring traffic
            scalar.wait_ge(dve_sem, 1)
            scalar.dma_start(out=y_minus, in_=t[:, :, :]).then_inc(st_sem, 16)
            scalar.wait_ge(st_sem, 16)

    return nc


def kernel(x: np.ndarray, **trace_kwargs) -> np.ndarray:
    from concourse.bass_utils import run_bass_kernel_spmd

    x = np.asarray(x, dtype=np.float32)
    if "nc" not in _cache:
        _cache["nc"] = _build_program()
        _cache["perms"] = _row_assignment()
    nc = _cache["nc"]
    perms = _cache["perms"]

    in_maps = [{"x": np.ascontiguousarray(x[perm])} for perm in perms]

    res = run_bass_kernel_spmd(
        nc, in_maps, core_ids=list(range(N_CORES)), **trace_kwargs
    )
    _cache["last_results"] = res

    y = np.empty((DIM, BATCH), dtype=np.float32)
    for perm, r in zip(perms, res.results):
        y[perm] = r["y"]
    return y


# revision 8
# speedup vs baseline: 1.6708x; 1.6708x over previous
"""CZ-ring (12 wires) applied to a batch of states: y = U @ x.

Every gate in the ring is a controlled-Z, which is diagonal in the
computational basis: CZ(c,t) = diag((-1)^(b_c & b_t)).  The product of
the 12 ring CZ gates is therefore also diagonal:

    U = diag(d),   d[b] = (-1)^(sum_i b_i * b_{(i+1) mod 12})

so U @ x is just a per-row sign flip of x.  Of the 4096 rows, 2112
have d=+1 and 1984 have d=-1.  Rows are sharded across the 8 cores
with a host-side permutation that gives every core the same layout:

    chunk 0 (rows   0..127): all "+"
    chunk 1 (rows 128..255): all "+"
    chunk 2 (rows 256..383): 8 "+" rows (pre-negated on host), 120 "-"
    chunk 3 (rows 384..511): all "-"

On device each 512 KiB chunk streams HBM -> SBUF -> HBM on the SP
HWDGE ring (16-SDMA-engine fanout); chunks 2/3 get a whole-tile
vector-engine multiply by the immediate -1.0 before the store (the 8
stray "+" rows are negated on the host first so they come out
unchanged).  Each core moves 2 MiB in + 2 MiB out -> HBM-bound.
"""

import numpy as np

N_WIRES = 12
DIM = 1 << N_WIRES  # 4096
BATCH = 1024
N_CORES = 8
ROWS_PER_CORE = DIM // N_CORES  # 512
P = 128
N_CHUNKS = 4
PLUS_PER_CORE = 264  # 2112 / 8
MINUS_PER_CORE = 248  # 1984 / 8
MIXED_PLUS = PLUS_PER_CORE - 2 * P  # 8 "+" rows in chunk 2

_cache: dict = {}


def _sign_parity() -> np.ndarray:
    """parity[b] = sum_i b_i * b_{(i+1) mod N_WIRES} mod 2  (1 => d=-1)."""
    b = np.arange(DIM, dtype=np.uint32)
    parity = np.zeros(DIM, dtype=np.uint32)
    for i in range(N_WIRES):
        bi = (b >> np.uint32(i)) & np.uint32(1)
        bj = (b >> np.uint32((i + 1) % N_WIRES)) & np.uint32(1)
        parity ^= bi & bj
    return parity


def _row_assignment():
    """Per-core row index lists in the chunk layout documented above."""
    parity = _sign_parity()
    plus_rows = np.nonzero(parity == 0)[0]  # 2112
    minus_rows = np.nonzero(parity == 1)[0]  # 1984
    assert len(plus_rows) == PLUS_PER_CORE * N_CORES
    assert len(minus_rows) == MINUS_PER_CORE * N_CORES
    perms = []
    for k in range(N_CORES):
        p = plus_rows[k * PLUS_PER_CORE : (k + 1) * PLUS_PER_CORE]
        m = minus_rows[k * MINUS_PER_CORE : (k + 1) * MINUS_PER_CORE]
        perms.append(np.concatenate([p, m]))
    return perms


def _build_program():
    from concourse import bass
    import concourse.mybir as mybir

    f32 = mybir.dt.float32
    nc = bass.Bass("TRN2", target_bir_lowering=False, debug=False)
    x_in = nc.dram_tensor("x", [ROWS_PER_CORE, BATCH], f32, kind="ExternalInput").ap()
    y_out = nc.dram_tensor(
        "y", [ROWS_PER_CORE, BATCH], f32, kind="ExternalOutput"
    ).ap()
    tiles = [
        nc.alloc_sbuf_tensor(f"t{c}", [P, BATCH], f32).ap() for c in range(N_CHUNKS)
    ]

    # Raw bass (no TileContext): the tile scheduler's tail Drain collects one
    # sem wait per DMA lane + engine and overflows this toolchain's
    # per-instruction sync-wait budget; explicit standalone waits keep every
    # instruction at <=1 wait.
    # One semaphore per load: a shared cumulative counter would let incs
    # from a later load satisfy an earlier chunk's wait (the 16 SDMA
    # engines complete independently), racing the negate against the load.
    with (
        nc.Block() as block,
        nc.semaphore("ld0") as ld0,
        nc.semaphore("ld1") as ld1,
        nc.semaphore("ld2") as ld2,
        nc.semaphore("ld3") as ld3,
        nc.semaphore("st_sem") as st_sem,
        nc.semaphore("dve_sem") as dve_sem,
    ):
        ld_sems = [ld0, ld1, ld2, ld3]

        @block.sync
        def _(sync: bass.BassEngine):
            for c in range(N_CHUNKS):
                sync.dma_start(
                    out=tiles[c][:, :], in_=x_in[c * P : (c + 1) * P, :]
                ).then_inc(ld_sems[c], 16)
            # "+" chunks: store as soon as their load lands
            for c in (0, 1):
                sync.wait_ge(ld_sems[c], 16)
                sync.dma_start(
                    out=y_out[c * P : (c + 1) * P, :], in_=tiles[c][:, :]
                ).then_inc(st_sem, 16)
            # "-" chunks: store once the vector engine has negated them
            for i, c in enumerate((2, 3)):
                sync.wait_ge(dve_sem, i + 1)
                sync.dma_start(
                    out=y_out[c * P : (c + 1) * P, :], in_=tiles[c][:, :]
                ).then_inc(st_sem, 16)
            sync.wait_ge(st_sem, 16 * N_CHUNKS)

        @block.vector
        def _(vector: bass.BassEngine):
            # whole-tile negates; the 8 "+" rows in chunk 2 are pre-negated
            # on the host so they come out unchanged
            vector.wait_ge(ld2, 16)
            vector.tensor_scalar_mul(
                tiles[2][:, :], tiles[2][:, :], -1.0
            ).then_inc(dve_sem, 1)
            vector.wait_ge(ld3, 16)
            vector.tensor_scalar_mul(
                tiles[3][:, :], tiles[3][:, :], -1.0
            ).then_inc(dve_sem, 1)

    return nc


def kernel(x: np.ndarray, **trace_kwargs) -> np.ndarray:
    from concourse.bass_utils import run_bass_kernel_spmd

    x = np.asarray(x, dtype=np.float32)
    if "nc" not in _cache:
        _cache["nc"] = _build_program()
        _cache["perms"] = _row_assignment()
    nc = _cache["nc"]
    perms = _cache["perms"]

    in_maps = []
    for perm in perms:
        xs = np.ascontiguousarray(x[perm])
        # chunk 2 holds 8 "+" rows (shard positions 256..263); the device
        # negates chunks 2-3 wholesale, so pre-negate these to compensate
        xs[2 * P : 2 * P + MIXED_PLUS] *= -1.0
        in_maps.append({"x": xs})

    res = run_bass_kernel_spmd(
        nc, in_maps, core_ids=list(range(N_CORES)), **trace_kwargs
    )
    _cache["last_results"] = res

    y = np.empty((DIM, BATCH), dtype=np.float32)
    for perm, r in zip(perms, res.results):
        y[perm] = r["y"]
    return y
